# revision 2
# baseline (speedup 1.0000x reference)
"""Trainium2 Bass kernel for LowRankRayTracer.

csi[f] = (delta_t/D) * v_f^T M v_f,  M = conj(rad)^T conj(att)  (R=32, complex)
contracted over N = D*K = 524288 rows.

Strategy (8 cores):
  - Shard the N rows across cores (512 directions each). csi is linear in M,
    so each core computes its partial S = rad32^T att32 (64x64, f32 view of
    complex pairs -> all four real cross products at once), builds
    W = [W_real | W_imag] (block form), computes partial csi over ALL F=8192
    subcarriers, and the host just sums the 8 partial csi vectors.
  - fp32 matmul is 4 cyc/col on TRN2 PE, so inputs are split on the host into
    fp16 hi+lo (same total bytes); with the att hi/lo pair packed side by side
    as one 256-wide moving operand, two matmuls per slice (lhsT=rad_h, rad_l)
    produce all four products hh|hl|lh|ll -- exact reconstruction, and half
    the LDWEIGHTS of a 3-pass version (LDW is the PE bottleneck otherwise).
  - Matmuls accumulate round-robin into 4 PSUM banks (avoids same-bank RMW
    serialization); diagonal blocks summed later via selection matmuls.
"""

import numpy as np

D, K, R = 4096, 128, 32
F = 8192
N_CORES = 8
DIR_PER_CORE = D // N_CORES              # 512
ROWS_PER_CORE = DIR_PER_CORE * K         # 65536 rows of (64,) f32
N_MACRO = 8                              # macro tiles per tensor per core
MACRO_COLS = 4096                        # fp16 per partition per macro tile
SLICE = 128                              # matmul slice width (2 rows/partition)
SCALE = (200.0 / K) / D                  # delta_t / num_directions (exact binary)
FCHUNK = 512                             # phase-3 subcarriers per chunk
N_FCHUNK = F // FCHUNK                   # 16
NB = 4                                   # round-robin PSUM accumulator banks

_NC_CACHE = {}


def _build_consts():
    """(128, 258) f32: four (128,64) selection matrices + ones-selector cols."""
    c = np.zeros((128, 258), np.float32)
    EA = np.zeros((128, 32), np.float32)
    OA = np.zeros((128, 32), np.float32)
    EB = np.zeros((128, 32), np.float32)
    OB = np.zeros((128, 32), np.float32)
    for m in range(32):
        EA[2 * m, m] = 1.0
        OA[2 * m + 1, m] = 1.0
        EB[64 + 2 * m, m] = 1.0
        OB[64 + 2 * m + 1, m] = 1.0
    c[:, 0:32] = EA
    c[:, 32:64] = OA
    c[:, 64:96] = EB
    c[:, 96:128] = OB
    c[:, 128:160] = OA
    c[:, 160:192] = EA
    c[:, 192:224] = OB
    c[:, 224:256] = EB
    c[0:64, 256] = 1.0
    c[64:128, 257] = 1.0
    return c


def build_nc(n_macro=N_MACRO):
    import concourse.bacc as bacc
    import concourse.mybir as mybir
    import concourse.tile as tile

    fp32 = mybir.dt.float32
    fp16 = mybir.dt.float16
    nc = bacc.Bacc(trn_type="TRN2", target_bir_lowering=False, debug=False)

    rad_h_d = nc.dram_tensor("rad_h", [n_macro, 128, MACRO_COLS], fp16,
                             kind="ExternalInput").ap()
    rad_l_d = nc.dram_tensor("rad_l", [n_macro, 128, MACRO_COLS], fp16,
                             kind="ExternalInput").ap()
    att_hl_d = nc.dram_tensor("att_hl", [n_macro, 128, 2 * MACRO_COLS], fp16,
                              kind="ExternalInput").ap()
    gtd_d = nc.dram_tensor("gtd", [128, F], fp32, kind="ExternalInput").ap()
    gth_d = nc.dram_tensor("gth", [64, F], fp16, kind="ExternalInput").ap()
    gtl_d = nc.dram_tensor("gtl", [64, F], fp16, kind="ExternalInput").ap()
    cst_d = nc.dram_tensor("consts", [128, 258], fp32, kind="ExternalInput").ap()
    out_d = nc.dram_tensor("csi", [2, F], fp32, kind="ExternalOutput").ap()

    with tile.TileContext(nc) as tc:
        with (
            tc.tile_pool(name="io", bufs=2) as io_pool,
            tc.tile_pool(name="small", bufs=1) as small,
            tc.tile_pool(name="epool", bufs=8) as epool,
        ):
            # constants up front (tiny); gtd issued after the main-loop DMAs
            # so it doesn't steal early HBM bandwidth (not needed till phase 3)
            c_sb = small.tile([128, 258], fp32, tag="consts")
            nc.sync.dma_start(c_sb[:], cst_d[:])
            gtd_sb = small.tile([128, F], fp32, tag="gtd")
            gth_sb = small.tile([64, F], fp16, tag="gth")
            gtl_sb = small.tile([64, F], fp16, tag="gtl")

            # ---- main loop: S += rad^T att via fp16 hi/lo, 256-wide rhs ----
            # lhsT=rad_h over rhs=[att_h|att_l] gives [hh|hl]; lhsT=rad_l
            # gives [lh|ll]. S = sum of all four 128-col blocks (exact).
            s_sb = small.tile([128, 128], fp32, tag="s_sb")
            n_slices = MACRO_COLS // SLICE
            total = n_macro * n_slices * 2
            with tc.tile_pool(name="spsum", bufs=1, space="PSUM") as spsum:
                banks = [spsum.tile([128, 2 * SLICE], fp32, tag=f"s{b}",
                                    name=f"sbank{b}")
                         for b in range(NB)]
                seen = [False] * NB
                idx = 0
                for i in range(n_macro):
                    rad_h = io_pool.tile([128, MACRO_COLS], fp16, tag="rad_h")
                    rad_l = io_pool.tile([128, MACRO_COLS], fp16, tag="rad_l")
                    att_hl = io_pool.tile([128, 2 * MACRO_COLS], fp16,
                                          tag="att_hl")
                    if i == 0:
                        # halve the first loads so the first matmuls start
                        # as soon as ~1.5 MiB has landed, not 4 MiB
                        hm = MACRO_COLS // 2
                        nc.sync.dma_start(rad_h[:, 0:hm], rad_h_d[0, :, 0:hm])
                        nc.scalar.dma_start(att_hl[:, 0:2 * hm],
                                            att_hl_d[0, :, 0:2 * hm])
                        nc.sync.dma_start(rad_l[:, 0:hm], rad_l_d[0, :, 0:hm])
                        nc.sync.dma_start(rad_h[:, hm:], rad_h_d[0, :, hm:])
                        nc.scalar.dma_start(att_hl[:, 2 * hm:],
                                            att_hl_d[0, :, 2 * hm:])
                        nc.sync.dma_start(rad_l[:, hm:], rad_l_d[0, :, hm:])
                    else:
                        nc.sync.dma_start(rad_h[:], rad_h_d[i, :, :])
                        nc.sync.dma_start(rad_l[:], rad_l_d[i, :, :])
                        nc.scalar.dma_start(att_hl[:], att_hl_d[i, :, :])
                    for s in range(n_slices):
                        rsl = slice(s * SLICE, (s + 1) * SLICE)
                        asl = slice(s * 2 * SLICE, (s + 1) * 2 * SLICE)
                        for lh in (rad_h, rad_l):
                            b = idx % NB
                            nc.tensor.matmul(
                                banks[b][:],
                                lhsT=lh[:, rsl],
                                rhs=att_hl[:, asl],
                                start=not seen[b],
                                stop=(idx >= total - NB),
                            )
                            seen[b] = True
                            idx += 1

                nc.sync.dma_start(gtd_sb[:], gtd_d[:])
                nc.sync.dma_start(gth_sb[:], gth_d[:])
                nc.sync.dma_start(gtl_sb[:], gtl_d[:])

                # S = sum of all four 128-col blocks over the 4 banks
                acc = small.tile([128, 2 * SLICE], fp32, tag="acc")
                nc.vector.tensor_copy(acc[:], banks[0][:])
                for b in range(1, NB):
                    nc.vector.tensor_add(acc[:], acc[:], banks[b][:])
                nc.vector.tensor_add(s_sb[:], acc[:, 0:SLICE],
                                     acc[:, SLICE:2 * SLICE])

            # ---- epilogue: build W = [W_real | W_imag] (64, 128) ----
            with tc.tile_pool(name="vpsum", bufs=1, space="PSUM") as vpsum:
                v1 = vpsum.tile([64, 64], fp32, tag="v1")
                nc.tensor.matmul(v1[:], lhsT=c_sb[:, 0:64], rhs=s_sb[:, 0:64],
                                 start=True, stop=False)
                nc.tensor.matmul(v1[:], lhsT=c_sb[:, 64:128],
                                 rhs=s_sb[:, 64:128], start=False, stop=True)
                v2 = vpsum.tile([64, 64], fp32, tag="v2")
                nc.tensor.matmul(v2[:], lhsT=c_sb[:, 128:192],
                                 rhs=s_sb[:, 0:64], start=True, stop=False)
                nc.tensor.matmul(v2[:], lhsT=c_sb[:, 192:256],
                                 rhs=s_sb[:, 64:128], start=False, stop=True)

                v1s = small.tile([64, 64], fp32, tag="v1s")
                nc.vector.tensor_copy(v1s[:], v1[:])
                v2s = small.tile([64, 64], fp32, tag="v2s")
                nc.vector.tensor_copy(v2s[:], v2[:])

            # mr = Mr (dup-stacked), mp = -Mi (dup-stacked)
            mr = small.tile([64, 32], fp32, tag="mr")
            mp = small.tile([64, 32], fp32, tag="mp")
            nc.vector.tensor_sub(mr[0:32, :], v1s[0:32, 0:64:2], v2s[0:32, 1:64:2])
            nc.vector.tensor_sub(mr[32:64, :], v2s[32:64, 0:64:2], v1s[32:64, 1:64:2])
            nc.vector.tensor_add(mp[0:32, :], v1s[0:32, 1:64:2], v2s[0:32, 0:64:2])
            nc.vector.tensor_add(mp[32:64, :], v2s[32:64, 1:64:2], v1s[32:64, 0:64:2])

            wri = small.tile([64, 128], fp32, tag="wri")
            s_ = float(SCALE)
            # W_real = [[Mr, -Mi], [-Mi, -Mr]] * s
            nc.scalar.mul(wri[0:32, 0:32], mr[0:32, :], s_)
            nc.scalar.mul(wri[0:32, 32:64], mp[0:32, :], s_)
            nc.scalar.mul(wri[32:64, 0:32], mp[32:64, :], s_)
            nc.scalar.mul(wri[32:64, 32:64], mr[32:64, :], -s_)
            # W_imag = [[Mi, Mr], [Mr, -Mi]] * s
            nc.scalar.mul(wri[0:32, 64:96], mp[0:32, :], -s_)
            nc.scalar.mul(wri[0:32, 96:128], mr[0:32, :], s_)
            nc.scalar.mul(wri[32:64, 64:96], mr[32:64, :], s_)
            nc.scalar.mul(wri[32:64, 96:128], mp[32:64, :], s_)

            # fp16 hi/lo split of W for the phase-3 matmuls
            wh = small.tile([64, 128], fp16, tag="wh")
            nc.vector.tensor_copy(wh[:], wri[:])
            whf = small.tile([64, 128], fp32, tag="whf")
            nc.vector.tensor_copy(whf[:], wh[:])
            wlf = small.tile([64, 128], fp32, tag="wlf")
            nc.vector.tensor_sub(wlf[:], wri[:], whf[:])
            wl = small.tile([64, 128], fp16, tag="wl")
            nc.vector.tensor_copy(wl[:], wlf[:])

            # PE warm-keepers: cheap matmuls dependent on s_sb bridge the
            # epilogue gap so HAM doesn't re-throttle before phase 3
            with tc.tile_pool(name="wpsum", bufs=1, space="PSUM") as wpsum:
                warm_ps = wpsum.tile([64, 64], fp32, tag="warm")
                for w in range(10):
                    nc.tensor.matmul(warm_ps[:], lhsT=c_sb[:, 0:64],
                                     rhs=s_sb[:, 0:64], start=True, stop=True)

            # ---- phase 3: csi chunks over F ----
            # All T matmuls issued first so the per-chunk csi matmuls don't
            # head-of-line-block them in the in-order PE queue.
            csi_sb = small.tile([2, F], fp32, tag="csi_sb")
            with (
                tc.tile_pool(name="tpsum", bufs=6, space="PSUM") as tpsum,
                tc.tile_pool(name="cpsum", bufs=2, space="PSUM") as cpsum,
            ):
                t_tiles = []
                e_tiles = []
                for ci in range(N_FCHUNK):
                    fs = slice(ci * FCHUNK, (ci + 1) * FCHUNK)
                    t_ps = tpsum.tile([128, FCHUNK], fp32, tag="t",
                                      name=f"t{ci}")
                    # T = W^T g via fp16 hi/lo (dropped Wl*gl ~ 2^-22)
                    nc.tensor.matmul(t_ps[:], lhsT=wh[:], rhs=gth_sb[:, fs],
                                     start=True, stop=False)
                    nc.tensor.matmul(t_ps[:], lhsT=wl[:], rhs=gth_sb[:, fs],
                                     start=False, stop=False)
                    nc.tensor.matmul(t_ps[:], lhsT=wh[:], rhs=gtl_sb[:, fs],
                                     start=False, stop=True)
                    t_tiles.append(t_ps)
                    e_sb = epool.tile([128, FCHUNK], fp32, tag="e",
                                      name=f"e{ci}")
                    nc.vector.tensor_mul(e_sb[:], gtd_sb[:, fs], t_ps[:])
                    e_tiles.append(e_sb)
                for ci in range(N_FCHUNK):
                    fs = slice(ci * FCHUNK, (ci + 1) * FCHUNK)
                    c_ps = cpsum.tile([2, FCHUNK], fp32, tag="c",
                                      name=f"c{ci}")
                    nc.tensor.matmul(c_ps[:], lhsT=c_sb[:, 256:258],
                                     rhs=e_tiles[ci][:], start=True, stop=True)
                    nc.scalar.copy(csi_sb[:, fs], c_ps[:])

            nc.sync.dma_start(out_d[:], csi_sb[:])

    nc.compile()
    return nc


def _prep_shared(fbv):
    """gtd (128,F) f32 dup + fp16 hi/lo (64,F) from complex fbv (F, R)."""
    fbv32 = np.ascontiguousarray(fbv).view(np.float32).reshape(F, 2 * R)
    gbt = np.ascontiguousarray(
        np.concatenate([fbv32[:, 0::2].T, fbv32[:, 1::2].T], axis=0))
    gtd = np.ascontiguousarray(np.concatenate([gbt, gbt], axis=0))
    gth = gbt.astype(np.float16)
    gtl = (gbt - gth.astype(np.float32)).astype(np.float16)
    return gtd, gth, gtl


def _shard_hl(arr, core):
    """Core's complex64 shard -> (hi, lo) fp16 arrays (N_MACRO,128,MACRO_COLS)."""
    sh = arr[core * DIR_PER_CORE:(core + 1) * DIR_PER_CORE]
    f32 = np.ascontiguousarray(sh).view(np.float32).ravel()
    h = f32.astype(np.float16)
    lo = (f32 - h.astype(np.float32)).astype(np.float16)
    shp = (N_MACRO, 128, MACRO_COLS)
    return h.reshape(shp), lo.reshape(shp)


def _pack_hl(h, lo):
    """Interleave hi/lo at 128-col slice granularity: [...,s*256:+256] =
    [h_slice(128) | lo_slice(128)] -> (N_MACRO, 128, 2*MACRO_COLS)."""
    ns = MACRO_COLS // SLICE
    h4 = h.reshape(N_MACRO, 128, ns, SLICE)
    l4 = lo.reshape(N_MACRO, 128, ns, SLICE)
    return np.ascontiguousarray(
        np.stack([h4, l4], axis=3).reshape(N_MACRO, 128, 2 * MACRO_COLS))


def _build_in_maps(attenuation_vectors, radiation_vectors,
                   frequency_basis_vectors):
    gtd, gth, gtl = _prep_shared(frequency_basis_vectors)
    consts = _build_consts()
    in_maps = []
    for c in range(N_CORES):
        rh, rl = _shard_hl(radiation_vectors, c)
        ah, al = _shard_hl(attenuation_vectors, c)
        in_maps.append({
            "rad_h": rh, "rad_l": rl,
            "att_hl": _pack_hl(ah, al),
            "gtd": gtd, "gth": gth, "gtl": gtl,
            "consts": consts,
        })
    return in_maps


def kernel(attenuation_vectors, radiation_vectors, frequency_basis_vectors):
    from concourse.bass_utils import run_bass_kernel_spmd

    if "nc" not in _NC_CACHE:
        _NC_CACHE["nc"] = build_nc()
    nc = _NC_CACHE["nc"]

    in_maps = _build_in_maps(attenuation_vectors, radiation_vectors,
                             frequency_basis_vectors)
    res = run_bass_kernel_spmd(nc, in_maps, core_ids=list(range(N_CORES)))
    acc = np.zeros((2, F), np.float64)
    for r in res.results:
        acc += r["csi"]
    return (acc[0] + 1j * acc[1]).astype(np.complex64)



# revision 4
# speedup vs baseline: 1.3902x; 1.3902x over previous
"""Trainium2 Bass kernel for LowRankRayTracer.

csi[f] = (delta_t/D) * v_f^T M v_f,  M = conj(rad)^T conj(att)  (R=32, complex)
contracted over N = D*K = 524288 rows.

Strategy (8 cores):
  - Shard the N rows across cores (512 directions each). csi is linear in M,
    so each core computes its partial S = rad^T att (f32 view of complex
    pairs -> all four real cross products at once), folds S into
    W = [W_real | W_imag], computes partial csi over ALL F=8192 subcarriers,
    and the host just sums the 8 partial csi vectors.
  - Inputs are cast to plain fp16 on the host (tolerance is 2e-2; fp16
    rounding lands ~5e-4 after the 524288-term stochastic accumulation).
    This halves HBM traffic vs an exact hi/lo split AND cuts PE work 3x:
    each 128-row slice is one LDWEIGHTS(128) + one 128-col matmul, i.e.
    1 PE cycle per contracted row.
  - Each matmul packs 2 rows per partition: lhsT = rad[:, s*128:+128]
    (2 rows of 64 per partition), rhs = att same slice. PSUM accumulates
    [128,128] where blocks (0:64,0:64) and (64:128,64:128) are the valid
    row_a*row_a and row_b*row_b partial sums (cross blocks are garbage,
    discarded by the fold). Matmuls round-robin over 4 PSUM banks.
"""

import numpy as np

D, K, R = 4096, 128, 32
F = 8192
N_CORES = 8
DIR_PER_CORE = D // N_CORES              # 512
N_MACRO = 8                              # macro tiles per tensor per core
MACRO_COLS = 4096                        # fp16 per partition per macro tile
SLICE = 128                              # matmul slice width (2 rows/partition)
SCALE = (200.0 / K) / D                  # delta_t / num_directions (exact binary)
FCHUNK = 512                             # phase-3 subcarriers per chunk
N_FCHUNK = F // FCHUNK                   # 16
NB = 4                                   # round-robin PSUM accumulator banks

_NC_CACHE = {}


def _build_consts():
    """(128, 258) f32 selection matrices.

    S64[r, f] := acc[r, f] + acc[64+r, 64+f] (fold of the two valid blocks).
    v1 = C1^T acc[:,0:64] + C2^T acc[:,64:128]: v1[0:32] = even rows of S64,
    v1[32:64] = odd rows. v2 (C1x/C2x) is the even/odd swap. Cols 256/257:
    ones selectors for the final re/im column-sum matmuls.
    """
    c = np.zeros((128, 258), np.float32)
    for a in range(32):
        c[2 * a, a] = 1.0                # C1: even rows -> partitions 0:32
        c[2 * a + 1, 32 + a] = 1.0       #     odd rows  -> partitions 32:64
        c[64 + 2 * a, 64 + a] = 1.0      # C2: same for the b-half of acc
        c[64 + 2 * a + 1, 64 + 32 + a] = 1.0
        c[2 * a, 128 + 32 + a] = 1.0     # C1x: swapped
        c[2 * a + 1, 128 + a] = 1.0
        c[64 + 2 * a, 192 + 32 + a] = 1.0
        c[64 + 2 * a + 1, 192 + a] = 1.0
    c[0:64, 256] = 1.0
    c[64:128, 257] = 1.0
    return c


def build_nc(n_macro=N_MACRO):
    import concourse.bacc as bacc
    import concourse.mybir as mybir
    import concourse.tile as tile

    fp32 = mybir.dt.float32
    fp16 = mybir.dt.float16
    nc = bacc.Bacc(trn_type="TRN2", target_bir_lowering=False, debug=False)

    rad_d = nc.dram_tensor("rad", [n_macro, 128, MACRO_COLS], fp16,
                           kind="ExternalInput").ap()
    att_d = nc.dram_tensor("att", [n_macro, 128, MACRO_COLS], fp16,
                           kind="ExternalInput").ap()
    gth_d = nc.dram_tensor("gth", [64, F], fp16, kind="ExternalInput").ap()
    cst_d = nc.dram_tensor("consts", [128, 258], fp32, kind="ExternalInput").ap()
    out_d = nc.dram_tensor("csi", [2, F], fp32, kind="ExternalOutput").ap()

    with tile.TileContext(nc) as tc:
        with (
            tc.tile_pool(name="io", bufs=2) as io_pool,
            tc.tile_pool(name="small", bufs=1) as small,
            tc.tile_pool(name="epool", bufs=8) as epool,
        ):
            c_sb = small.tile([128, 258], fp32, tag="consts")
            nc.sync.dma_start(c_sb[:], cst_d[:])
            gth2 = small.tile([128, F], fp16, tag="gth2")
            gtdf = small.tile([128, F], fp32, tag="gtdf")

            # ---- main loop: S += rad^T att, plain fp16, 128-wide slices ----
            acc = small.tile([128, 128], fp32, tag="acc")
            n_slices = MACRO_COLS // SLICE
            total = n_macro * n_slices
            with tc.tile_pool(name="spsum", bufs=1, space="PSUM") as spsum:
                # full-bank tiles so the 4 accumulators live in 4 banks
                banks = [spsum.tile([128, 512], fp32, tag=f"s{b}",
                                    name=f"sbank{b}")
                         for b in range(NB)]
                seen = [False] * NB
                idx = 0
                for i in range(n_macro):
                    rad = io_pool.tile([128, MACRO_COLS], fp16, tag="rad")
                    att = io_pool.tile([128, MACRO_COLS], fp16, tag="att")
                    if i == n_macro - 1:
                        # g loads land just before the last macro so phase 3
                        # never waits on them
                        nc.sync.dma_start(gth2[0:64, :], gth_d[:])
                        nc.scalar.dma_start(gth2[64:128, :], gth_d[:])
                    # quarter-split the first macro so matmuls start early;
                    # half-split the rest for smooth slice-level dependencies
                    nchunk = 4 if i == 0 else 2
                    cw = MACRO_COLS // nchunk
                    for q in range(nchunk):
                        cs = slice(q * cw, (q + 1) * cw)
                        nc.sync.dma_start(rad[:, cs], rad_d[i, :, cs])
                        nc.scalar.dma_start(att[:, cs], att_d[i, :, cs])
                    for s in range(n_slices):
                        sl = slice(s * SLICE, (s + 1) * SLICE)
                        b = idx % NB
                        nc.tensor.matmul(
                            banks[b][:, 0:128],
                            lhsT=rad[:, sl],
                            rhs=att[:, sl],
                            start=not seen[b],
                            stop=(idx >= total - NB),
                        )
                        seen[b] = True
                        idx += 1

                # duplicated g in f32 for the phase-3 elementwise multiply
                nc.vector.tensor_copy(gtdf[:], gth2[:])

                # acc = sum of the 4 banks
                nc.vector.tensor_copy(acc[:], banks[0][:, 0:128])
                for b in range(1, NB):
                    nc.vector.tensor_add(acc[:], acc[:], banks[b][:, 0:128])

            # ---- epilogue: fold + de-interleave via selection matmuls ----
            with tc.tile_pool(name="vpsum", bufs=1, space="PSUM") as vpsum:
                v1 = vpsum.tile([64, 64], fp32, tag="v1")
                nc.tensor.matmul(v1[:], lhsT=c_sb[:, 0:64], rhs=acc[:, 0:64],
                                 start=True, stop=False)
                nc.tensor.matmul(v1[:], lhsT=c_sb[:, 64:128],
                                 rhs=acc[:, 64:128], start=False, stop=True)
                v2 = vpsum.tile([64, 64], fp32, tag="v2")
                nc.tensor.matmul(v2[:], lhsT=c_sb[:, 128:192],
                                 rhs=acc[:, 0:64], start=True, stop=False)
                nc.tensor.matmul(v2[:], lhsT=c_sb[:, 192:256],
                                 rhs=acc[:, 64:128], start=False, stop=True)

                v1s = small.tile([64, 64], fp32, tag="v1s")
                nc.vector.tensor_copy(v1s[:], v1[:])
                v2s = small.tile([64, 64], fp32, tag="v2s")
                nc.vector.tensor_copy(v2s[:], v2[:])

            # mr = Mr (dup-stacked), mp = -Mi (dup-stacked)
            mr = small.tile([64, 32], fp32, tag="mr")
            mp = small.tile([64, 32], fp32, tag="mp")
            nc.vector.tensor_sub(mr[0:32, :], v1s[0:32, 0:64:2], v2s[0:32, 1:64:2])
            nc.vector.tensor_sub(mr[32:64, :], v2s[32:64, 0:64:2], v1s[32:64, 1:64:2])
            nc.vector.tensor_add(mp[0:32, :], v1s[0:32, 1:64:2], v2s[0:32, 0:64:2])
            nc.vector.tensor_add(mp[32:64, :], v2s[32:64, 1:64:2], v1s[32:64, 0:64:2])

            wri = small.tile([64, 128], fp32, tag="wri")
            s_ = float(SCALE)
            # W_real = [[Mr, -Mi], [-Mi, -Mr]] * s
            nc.scalar.mul(wri[0:32, 0:32], mr[0:32, :], s_)
            nc.scalar.mul(wri[0:32, 32:64], mp[0:32, :], s_)
            nc.scalar.mul(wri[32:64, 0:32], mp[32:64, :], s_)
            nc.scalar.mul(wri[32:64, 32:64], mr[32:64, :], -s_)
            # W_imag = [[Mi, Mr], [Mr, -Mi]] * s
            nc.scalar.mul(wri[0:32, 64:96], mp[0:32, :], -s_)
            nc.scalar.mul(wri[0:32, 96:128], mr[0:32, :], s_)
            nc.scalar.mul(wri[32:64, 64:96], mr[32:64, :], s_)
            nc.scalar.mul(wri[32:64, 96:128], mp[32:64, :], s_)

            # fp16 hi/lo split of W for the phase-3 matmuls
            wh = small.tile([64, 128], fp16, tag="wh")
            nc.vector.tensor_copy(wh[:], wri[:])
            whf = small.tile([64, 128], fp32, tag="whf")
            nc.vector.tensor_copy(whf[:], wh[:])
            wlf = small.tile([64, 128], fp32, tag="wlf")
            nc.vector.tensor_sub(wlf[:], wri[:], whf[:])
            wl = small.tile([64, 128], fp16, tag="wl")
            nc.vector.tensor_copy(wl[:], wlf[:])

            # PE warm-keepers: cheap matmuls dependent on acc bridge the
            # epilogue gap so the p-state doesn't drop before phase 3
            with tc.tile_pool(name="wpsum", bufs=1, space="PSUM") as wpsum:
                warm_ps = wpsum.tile([64, 64], fp32, tag="warm")
                for w in range(6):
                    nc.tensor.matmul(warm_ps[:], lhsT=c_sb[:, 0:64],
                                     rhs=acc[:, 0:64], start=True, stop=True)

            # ---- phase 3: csi chunks over F ----
            csi_sb = small.tile([2, F], fp32, tag="csi_sb")
            with (
                tc.tile_pool(name="tpsum", bufs=6, space="PSUM") as tpsum,
                tc.tile_pool(name="cpsum", bufs=2, space="PSUM") as cpsum,
            ):
                e_tiles = []
                for ci in range(N_FCHUNK):
                    fs = slice(ci * FCHUNK, (ci + 1) * FCHUNK)
                    t_ps = tpsum.tile([128, FCHUNK], fp32, tag="t",
                                      name=f"t{ci}")
                    # T = W^T g via fp16 hi/lo W (dropped g_lo ~ 2^-11 rel)
                    nc.tensor.matmul(t_ps[:], lhsT=wh[:], rhs=gth2[0:64, fs],
                                     start=True, stop=False)
                    nc.tensor.matmul(t_ps[:], lhsT=wl[:], rhs=gth2[0:64, fs],
                                     start=False, stop=True)
                    e_sb = epool.tile([128, FCHUNK], fp32, tag="e",
                                      name=f"e{ci}")
                    nc.vector.tensor_mul(e_sb[:], gtdf[:, fs], t_ps[:])
                    e_tiles.append(e_sb)
                for ci in range(N_FCHUNK):
                    fs = slice(ci * FCHUNK, (ci + 1) * FCHUNK)
                    c_ps = cpsum.tile([2, FCHUNK], fp32, tag="c",
                                      name=f"c{ci}")
                    nc.tensor.matmul(c_ps[:], lhsT=c_sb[:, 256:258],
                                     rhs=e_tiles[ci][:], start=True, stop=True)
                    nc.scalar.copy(csi_sb[:, fs], c_ps[:])

            nc.sync.dma_start(out_d[:], csi_sb[:])

    nc.compile()
    return nc


def _prep_g(fbv):
    """gth (64, F) fp16: [fbv_re.T; fbv_im.T] from complex fbv (F, R)."""
    fbv32 = np.ascontiguousarray(fbv).view(np.float32).reshape(F, 2 * R)
    gbt = np.ascontiguousarray(
        np.concatenate([fbv32[:, 0::2].T, fbv32[:, 1::2].T], axis=0))
    return gbt.astype(np.float16)


def _shard_h(arr, core):
    """Core's complex64 shard -> fp16 (N_MACRO, 128, MACRO_COLS)."""
    sh = arr[core * DIR_PER_CORE:(core + 1) * DIR_PER_CORE]
    f32 = np.ascontiguousarray(sh).view(np.float32)
    return f32.astype(np.float16).reshape(N_MACRO, 128, MACRO_COLS)


def _build_in_maps(attenuation_vectors, radiation_vectors,
                   frequency_basis_vectors):
    gth = _prep_g(frequency_basis_vectors)
    consts = _build_consts()
    in_maps = []
    for c in range(N_CORES):
        in_maps.append({
            "rad": _shard_h(radiation_vectors, c),
            "att": _shard_h(attenuation_vectors, c),
            "gth": gth,
            "consts": consts,
        })
    return in_maps


def kernel(attenuation_vectors, radiation_vectors, frequency_basis_vectors):
    from concourse.bass_utils import run_bass_kernel_spmd

    if "nc" not in _NC_CACHE:
        _NC_CACHE["nc"] = build_nc()
    nc = _NC_CACHE["nc"]

    in_maps = _build_in_maps(attenuation_vectors, radiation_vectors,
                             frequency_basis_vectors)
    res = run_bass_kernel_spmd(nc, in_maps, core_ids=list(range(N_CORES)))
    acc = np.zeros((2, F), np.float64)
    for r in res.results:
        acc += r["csi"]
    return (acc[0] + 1j * acc[1]).astype(np.complex64)


# revision 13
# speedup vs baseline: 1.7176x; 1.2355x over previous
"""Trainium2 Bass kernel for LowRankRayTracer.

csi[f] = (delta_t/D) * v_f^T M v_f,  M = conj(rad)^T conj(att)  (R=32, complex)
contracted over N = D*K = 524288 rows.

Strategy (8 cores):
  - Shard the N rows across cores (512 directions each). csi is linear in M,
    so each core computes its partial S = rad^T att (f32 view of complex
    pairs -> all four real cross products at once), folds S into
    W = [W_real | W_imag], computes partial csi over ALL F=8192 subcarriers,
    and the host just sums the 8 partial csi vectors.
  - Inputs are cast to plain fp16 on the host (tolerance is 2e-2; fp16
    rounding lands ~5e-4 after the 524288-term stochastic accumulation).
    This halves HBM traffic vs an exact hi/lo split AND cuts PE work 3x:
    each 128-row slice is one LDWEIGHTS(128) + one 128-col matmul.
  - Each matmul packs 2 rows per partition: lhsT = rad[:, s*128:+128]
    (2 rows of 64 per partition), rhs = att same slice. PSUM accumulates
    [128,128] where blocks (0:64,0:64) and (64:128,64:128) are the valid
    row_a*row_a and row_b*row_b partial sums (cross blocks are garbage,
    discarded by the fold). Matmuls round-robin over 4 PSUM banks.
  - sqrt(delta_t/D) = 5/256 exactly, folded into g on the host, so W needs
    no scaling pass. W is built directly in fp16 by 8 strided DVE/Pool ops.
    Phase 3 keeps everything fp16 (1 cyc/col on the PE; f32 rhs would run
    at 4 cyc/col as two half-speed passes). csi chunks DMA straight from
    PSUM to DRAM.
"""

import numpy as np

D, K, R = 4096, 128, 32
F = 8192
N_CORES = 8
DIR_PER_CORE = D // N_CORES              # 512
N_MACRO = 8                              # macro tiles per tensor per core
MACRO_COLS = 4096                        # fp16 per partition per macro tile
SLICE = 128                              # matmul slice width (2 rows/partition)
SCALE = (200.0 / K) / D                  # delta_t / num_directions
GSCALE = 5.0 / 256.0                     # exact sqrt(SCALE)
FCHUNK = 512                             # phase-3 subcarriers per chunk
N_FCHUNK = F // FCHUNK                   # 16
NB = 4                                   # round-robin PSUM accumulator banks

_NC_CACHE = {}


def _build_consts():
    """(128, 256) f32 selection matrices.

    S64[r, f] := acc[r, f] + acc[64+r, 64+f] (fold of the two valid blocks).
    v1 = C1^T acc[:,0:64] + C2^T acc[:,64:128]: v1[0:32] = even rows of S64,
    v1[32:64] = odd rows. v2 (C1x/C2x) is the even/odd swap.
    """
    c = np.zeros((128, 256), np.float32)
    for a in range(32):
        c[2 * a, a] = 1.0                # C1: even rows -> partitions 0:32
        c[2 * a + 1, 32 + a] = 1.0       #     odd rows  -> partitions 32:64
        c[64 + 2 * a, 64 + a] = 1.0      # C2: same for the b-half of acc
        c[64 + 2 * a + 1, 64 + 32 + a] = 1.0
        c[2 * a, 128 + 32 + a] = 1.0     # C1x: swapped
        c[2 * a + 1, 128 + a] = 1.0
        c[64 + 2 * a, 192 + 32 + a] = 1.0
        c[64 + 2 * a + 1, 192 + a] = 1.0
    return c


def build_nc(n_macro=N_MACRO):
    import concourse.bacc as bacc
    import concourse.mybir as mybir
    import concourse.tile as tile

    fp32 = mybir.dt.float32
    fp16 = mybir.dt.float16
    mult = mybir.AluOpType.mult
    sub_ = mybir.AluOpType.subtract
    nc = bacc.Bacc(trn_type="TRN2", target_bir_lowering=False, debug=False)

    rad_d = nc.dram_tensor("rad", [n_macro, 128, MACRO_COLS], fp16,
                           kind="ExternalInput").ap()
    att_d = nc.dram_tensor("att", [n_macro, 128, MACRO_COLS], fp16,
                           kind="ExternalInput").ap()
    gth_d = nc.dram_tensor("gth", [64, F], fp16, kind="ExternalInput").ap()
    cst_d = nc.dram_tensor("consts", [128, 256], fp32, kind="ExternalInput").ap()
    out_d = nc.dram_tensor("eout", [128, F], fp16, kind="ExternalOutput").ap()

    # main-loop streaming plan: small first tiles so matmuls start early
    chunks = [(0, 1024), (1024, 1024), (2048, 1024), (3072, 1024)]
    chunks += [(i * MACRO_COLS, MACRO_COLS) for i in range(1, n_macro)]
    total = (MACRO_COLS // SLICE) * n_macro                    # 256 slices

    with tile.TileContext(nc) as tc:
        with (
            tc.tile_pool(name="io_s", bufs=2) as io_s,
            tc.tile_pool(name="io", bufs=3) as io_pool,
            tc.tile_pool(name="small", bufs=1) as small,
            tc.tile_pool(name="epool", bufs=8) as epool,
            tc.tile_pool(name="tsb", bufs=3) as tsb_pool,
        ):
            c_sb = small.tile([128, 256], fp32, tag="consts")
            nc.sync.dma_start(c_sb[:], cst_d[:])
            gth2 = small.tile([128, F], fp16, tag="gth2")

            # ---- main loop: S += rad^T att, plain fp16, 128-wide slices ----
            acc = small.tile([128, 128], fp32, tag="acc")
            with tc.tile_pool(name="spsum", bufs=1, space="PSUM") as spsum:
                # full-bank tiles so the 4 accumulators live in 4 banks
                banks = [spsum.tile([128, 512], fp32, tag=f"s{b}",
                                    name=f"sbank{b}")
                         for b in range(NB)]
                seen = [False] * NB
                idx = 0
                for ci, (c0, w) in enumerate(chunks):
                    if ci == len(chunks) - 1:
                        # g lands just before the last macro streams in
                        nc.sync.dma_start(gth2[0:64, :], gth_d[:])
                        nc.scalar.dma_start(gth2[64:128, :], gth_d[:])
                    pool = io_s if w < MACRO_COLS else io_pool
                    rad = pool.tile([128, w], fp16, tag=f"rad{w}")
                    att = pool.tile([128, w], fp16, tag=f"att{w}")
                    mi, off = divmod(c0, MACRO_COLS)
                    nc.sync.dma_start(rad[:], rad_d[mi, :, off:off + w])
                    nc.scalar.dma_start(att[:], att_d[mi, :, off:off + w])
                    for s in range(w // SLICE):
                        sl = slice(s * SLICE, (s + 1) * SLICE)
                        b = idx % NB
                        nc.tensor.matmul(
                            banks[b][:, 0:128],
                            lhsT=rad[:, sl],
                            rhs=att[:, sl],
                            start=not seen[b],
                            stop=(idx >= total - NB),
                        )
                        seen[b] = True
                        idx += 1

                # acc = sum of the 4 banks
                nc.vector.tensor_copy(acc[:], banks[0][:, 0:128])
                for b in range(1, NB):
                    nc.vector.tensor_add(acc[:], acc[:], banks[b][:, 0:128])

            # ---- epilogue: fold + de-interleave via selection matmuls ----
            with tc.tile_pool(name="vpsum", bufs=1, space="PSUM") as vpsum:
                v1 = vpsum.tile([64, 64], fp32, tag="v1")
                nc.tensor.matmul(v1[:], lhsT=c_sb[:, 0:64], rhs=acc[:, 0:64],
                                 start=True, stop=False)
                nc.tensor.matmul(v1[:], lhsT=c_sb[:, 64:128],
                                 rhs=acc[:, 64:128], start=False, stop=True)
                v2 = vpsum.tile([64, 64], fp32, tag="v2")
                nc.tensor.matmul(v2[:], lhsT=c_sb[:, 128:192],
                                 rhs=acc[:, 0:64], start=True, stop=False)
                nc.tensor.matmul(v2[:], lhsT=c_sb[:, 192:256],
                                 rhs=acc[:, 64:128], start=False, stop=True)

                v1s = small.tile([64, 64], fp32, tag="v1s")
                nc.vector.tensor_copy(v1s[:], v1[:])
                v2s = small.tile([64, 64], fp32, tag="v2s")
                nc.vector.tensor_copy(v2s[:], v2[:])

            # ---- build W = [W_real | W_imag] (64,128) directly in fp16 ----
            # Mr[a,b] = S64[2a,2b]-S64[2a+1,2b+1], Mi = -(S64[2a,2b+1]+S64[2a+1,2b])
            # W_real = [[Mr, -Mi], [-Mi, -Mr]], W_imag = [[Mi, Mr], [Mr, -Mi]]
            # v1[0:32]=even rows, v1[32:64]=odd; v2 swapped. Scale is folded
            # into g on the host (GSCALE^2 == SCALE).
            wh = small.tile([64, 128], fp16, tag="wh")
            E, O = slice(0, 64, 2), slice(1, 64, 2)
            t, b = slice(0, 32), slice(32, 64)
            # top rows: Mr | -Mi(=mp) | Mi | Mr
            nc.vector.tensor_sub(wh[t, 0:32], v1s[t, E], v2s[t, O])
            nc.vector.tensor_add(wh[t, 32:64], v1s[t, O], v2s[t, E])
            nc.vector.scalar_tensor_tensor(wh[t, 64:96], v1s[t, O], -1.0,
                                           v2s[t, E], op0=mult, op1=sub_)
            nc.gpsimd.tensor_sub(wh[t, 96:128], v1s[t, E], v2s[t, O])
            # bottom rows: -Mi(=mp) | -Mr | Mr | -Mi(=mp)
            nc.vector.tensor_add(wh[b, 0:32], v2s[b, O], v1s[b, E])
            nc.vector.tensor_sub(wh[b, 32:64], v1s[b, O], v2s[b, E])
            nc.gpsimd.tensor_sub(wh[b, 64:96], v2s[b, E], v1s[b, O])
            nc.gpsimd.tensor_add(wh[b, 96:128], v2s[b, O], v1s[b, E])

            # PE warm-keepers bridge the epilogue gap
            with tc.tile_pool(name="wpsum", bufs=1, space="PSUM") as wpsum:
                warm_ps = wpsum.tile([64, 64], fp32, tag="warm")
                for w in range(4):
                    nc.tensor.matmul(warm_ps[:], lhsT=c_sb[:, 0:64],
                                     rhs=acc[:, 0:64], start=True, stop=True)

            # ---- phase 3: e = g .* (W^T g) chunks stream straight to DRAM;
            # the host does the final (tiny) column sums ----
            with tc.tile_pool(name="tpsum", bufs=6, space="PSUM") as tpsum:
                for ci in range(N_FCHUNK):
                    fs = slice(ci * FCHUNK, (ci + 1) * FCHUNK)
                    t_ps = tpsum.tile([128, FCHUNK], fp32, tag="t",
                                      name=f"t{ci}")
                    nc.tensor.matmul(t_ps[:], lhsT=wh[:], rhs=gth2[0:64, fs],
                                     start=True, stop=True)
                    e_sb = epool.tile([128, FCHUNK], fp16, tag="e",
                                      name=f"e{ci}")
                    if ci % 2 == 0:
                        nc.vector.tensor_mul(e_sb[:], gth2[:, fs], t_ps[:])
                    else:
                        # stage T to SBUF on Act so the Pool engine (no PSUM
                        # access) can handle half the elementwise work
                        t_sb = tsb_pool.tile([128, FCHUNK], fp16, tag="tsb",
                                             name=f"tsb{ci}")
                        nc.scalar.copy(t_sb[:], t_ps[:])
                        nc.gpsimd.tensor_mul(e_sb[:], gth2[:, fs], t_sb[:])
                    nc.sync.dma_start(out_d[:, fs], e_sb[:])

    nc.compile()
    return nc


def _prep_g(fbv):
    """gth (64, F) fp16: sqrt(SCALE) * [fbv_re.T; fbv_im.T]."""
    fbv32 = np.ascontiguousarray(fbv).view(np.float32).reshape(F, 2 * R)
    gbt = np.concatenate([fbv32[:, 0::2].T, fbv32[:, 1::2].T], axis=0)
    return (gbt * np.float32(GSCALE)).astype(np.float16)


def _shard_h(arr, core):
    """Core's complex64 shard -> fp16 (N_MACRO, 128, MACRO_COLS)."""
    sh = arr[core * DIR_PER_CORE:(core + 1) * DIR_PER_CORE]
    f32 = np.ascontiguousarray(sh).view(np.float32)
    return f32.astype(np.float16).reshape(N_MACRO, 128, MACRO_COLS)


def _build_in_maps(attenuation_vectors, radiation_vectors,
                   frequency_basis_vectors):
    gth = _prep_g(frequency_basis_vectors)
    consts = _build_consts()
    in_maps = []
    for c in range(N_CORES):
        in_maps.append({
            "rad": _shard_h(radiation_vectors, c),
            "att": _shard_h(attenuation_vectors, c),
            "gth": gth,
            "consts": consts,
        })
    return in_maps


def kernel(attenuation_vectors, radiation_vectors, frequency_basis_vectors):
    from concourse.bass_utils import run_bass_kernel_spmd

    if "nc" not in _NC_CACHE:
        _NC_CACHE["nc"] = build_nc()
    nc = _NC_CACHE["nc"]

    in_maps = _build_in_maps(attenuation_vectors, radiation_vectors,
                             frequency_basis_vectors)
    res = run_bass_kernel_spmd(nc, in_maps, core_ids=list(range(N_CORES)))
    etot = np.zeros((128, F), np.float64)
    for r in res.results:
        etot += r["eout"].astype(np.float64)
    return (etot[0:64].sum(axis=0)
            + 1j * etot[64:128].sum(axis=0)).astype(np.complex64)


# revision 16
# speedup vs baseline: 1.8796x; 1.0943x over previous
"""Trainium2 Bass kernel for LowRankRayTracer.

csi[f] = (delta_t/D) * v_f^T M v_f,  M = conj(rad)^T conj(att)  (R=32, complex)
contracted over N = D*K = 524288 rows.

Strategy (8 cores):
  - Shard the N rows across cores (512 directions each). csi is linear in M,
    so each core computes its partial S = rad^T att (f32 view of complex
    pairs -> all four real cross products at once), folds S into
    W = [W_real | W_imag], computes partial csi over ALL F=8192 subcarriers,
    and the host just sums the 8 partial csi vectors.
  - Inputs are cast to plain fp16 on the host (tolerance is 2e-2; fp16
    rounding lands ~5e-4 after the 524288-term stochastic accumulation).
    This halves HBM traffic vs an exact hi/lo split AND cuts PE work 3x:
    each 128-row slice is one LDWEIGHTS(128) + one 128-col matmul.
  - Each matmul packs 2 rows per partition: lhsT = rad[:, s*128:+128]
    (2 rows of 64 per partition), rhs = att same slice. PSUM accumulates
    [128,128] where blocks (0:64,0:64) and (64:128,64:128) are the valid
    row_a*row_a and row_b*row_b partial sums (cross blocks are garbage,
    discarded by the fold). Matmuls round-robin over 4 PSUM banks.
  - sqrt(delta_t/D) = 5/256 exactly, folded into g on the host, so W needs
    no scaling pass. W is built directly in fp16 by 8 strided DVE/Pool ops.
    Phase 3 keeps everything fp16 (1 cyc/col on the PE; f32 rhs would run
    at 4 cyc/col as two half-speed passes). csi chunks DMA straight from
    PSUM to DRAM.
"""

import numpy as np

D, K, R = 4096, 128, 32
F = 8192
N_CORES = 8
DIR_PER_CORE = D // N_CORES              # 512
N_MACRO = 8                              # macro tiles per tensor per core
MACRO_COLS = 4096                        # fp16 per partition per macro tile
SLICE = 128                              # matmul slice width (2 rows/partition)
SCALE = (200.0 / K) / D                  # delta_t / num_directions
GSCALE = 5.0 / 256.0                     # exact sqrt(SCALE)
FCHUNK = 512                             # phase-3 subcarriers per chunk
N_FCHUNK = F // FCHUNK                   # 16
NB = 4                                   # round-robin PSUM accumulator banks

_NC_CACHE = {}


def _build_consts():
    """(128, 256) f32 selection matrices.

    S64[r, f] := acc[r, f] + acc[64+r, 64+f] (fold of the two valid blocks).
    v1 = C1^T acc[:,0:64] + C2^T acc[:,64:128]: v1[0:32] = even rows of S64,
    v1[32:64] = odd rows. v2 (C1x/C2x) is the even/odd swap.
    """
    c = np.zeros((128, 256), np.float32)
    for a in range(32):
        c[2 * a, a] = 1.0                # C1: even rows -> partitions 0:32
        c[2 * a + 1, 32 + a] = 1.0       #     odd rows  -> partitions 32:64
        c[64 + 2 * a, 64 + a] = 1.0      # C2: same for the b-half of acc
        c[64 + 2 * a + 1, 64 + 32 + a] = 1.0
        c[2 * a, 128 + 32 + a] = 1.0     # C1x: swapped
        c[2 * a + 1, 128 + a] = 1.0
        c[64 + 2 * a, 192 + 32 + a] = 1.0
        c[64 + 2 * a + 1, 192 + a] = 1.0
    return c


def build_nc(n_macro=N_MACRO):
    import concourse.bacc as bacc
    import concourse.mybir as mybir
    import concourse.tile as tile

    fp32 = mybir.dt.float32
    fp16 = mybir.dt.float16
    mult = mybir.AluOpType.mult
    sub_ = mybir.AluOpType.subtract
    nc = bacc.Bacc(trn_type="TRN2", target_bir_lowering=False, debug=False)

    rad_d = nc.dram_tensor("rad", [4, 128, 2 * MACRO_COLS], fp16,
                           kind="ExternalInput").ap()
    att_d = nc.dram_tensor("att", [4, 128, 2 * MACRO_COLS], fp16,
                           kind="ExternalInput").ap()
    gth_d = nc.dram_tensor("gth", [64, F], fp16, kind="ExternalInput").ap()
    cst_d = nc.dram_tensor("consts", [128, 256], fp32, kind="ExternalInput").ap()
    out_d = nc.dram_tensor("eout", [128, F], fp16, kind="ExternalOutput").ap()

    # main-loop streaming plan over the [4, 128, 8192] layout: small first
    # tiles so matmuls start early, then full 2 MiB tiles (16 KB/partition
    # descriptors). First two chunks' att loads go on SP too — the Act
    # engine's preamble ACT_TABLE_LOAD would delay them ~8 us at startup.
    chunks = [(0, 0, 2048), (0, 2048, 2048), (0, 4096, 4096),
              (1, 0, 8192), (2, 0, 8192), (3, 0, 8192)]
    total = (MACRO_COLS // SLICE) * n_macro                    # 256 slices

    with tile.TileContext(nc) as tc:
        with (
            tc.tile_pool(name="io_s", bufs=2) as io_s,
            tc.tile_pool(name="io_m", bufs=1) as io_m,
            tc.tile_pool(name="io", bufs=3) as io_pool,
            tc.tile_pool(name="small", bufs=1) as small,
            tc.tile_pool(name="epool", bufs=8) as epool,
            tc.tile_pool(name="tsb", bufs=3) as tsb_pool,
        ):
            c_sb = small.tile([128, 256], fp32, tag="consts")
            nc.sync.dma_start(c_sb[:], cst_d[:])
            gth2 = small.tile([128, F], fp16, tag="gth2")

            # ---- main loop: S += rad^T att, plain fp16, 128-wide slices ----
            acc = small.tile([128, 128], fp32, tag="acc")
            with tc.tile_pool(name="spsum", bufs=1, space="PSUM") as spsum:
                # full-bank tiles so the 4 accumulators live in 4 banks
                banks = [spsum.tile([128, 512], fp32, tag=f"s{b}",
                                    name=f"sbank{b}")
                         for b in range(NB)]
                seen = [False] * NB
                idx = 0
                for ci, (ti, c0, w) in enumerate(chunks):
                    if ci == len(chunks) - 1:
                        # g lands just before the last tile streams in
                        nc.sync.dma_start(gth2[0:64, :], gth_d[:])
                        nc.scalar.dma_start(gth2[64:128, :], gth_d[:])
                    pool = {2048: io_s, 4096: io_m, 8192: io_pool}[w]
                    rad = pool.tile([128, w], fp16, tag=f"rad{w}")
                    att = pool.tile([128, w], fp16, tag=f"att{w}")
                    nc.sync.dma_start(rad[:], rad_d[ti, :, c0:c0 + w])
                    att_eng = nc.sync if ci < 2 else nc.scalar
                    att_eng.dma_start(att[:], att_d[ti, :, c0:c0 + w])
                    for s in range(w // SLICE):
                        sl = slice(s * SLICE, (s + 1) * SLICE)
                        b = idx % NB
                        nc.tensor.matmul(
                            banks[b][:, 0:128],
                            lhsT=rad[:, sl],
                            rhs=att[:, sl],
                            start=not seen[b],
                            stop=(idx >= total - NB),
                        )
                        seen[b] = True
                        idx += 1

                # acc = sum of the 4 banks
                nc.vector.tensor_copy(acc[:], banks[0][:, 0:128])
                for b in range(1, NB):
                    nc.vector.tensor_add(acc[:], acc[:], banks[b][:, 0:128])

            # ---- epilogue: fold + de-interleave via selection matmuls ----
            with tc.tile_pool(name="vpsum", bufs=1, space="PSUM") as vpsum:
                v1 = vpsum.tile([64, 64], fp32, tag="v1")
                nc.tensor.matmul(v1[:], lhsT=c_sb[:, 0:64], rhs=acc[:, 0:64],
                                 start=True, stop=False)
                nc.tensor.matmul(v1[:], lhsT=c_sb[:, 64:128],
                                 rhs=acc[:, 64:128], start=False, stop=True)
                v2 = vpsum.tile([64, 64], fp32, tag="v2")
                nc.tensor.matmul(v2[:], lhsT=c_sb[:, 128:192],
                                 rhs=acc[:, 0:64], start=True, stop=False)
                nc.tensor.matmul(v2[:], lhsT=c_sb[:, 192:256],
                                 rhs=acc[:, 64:128], start=False, stop=True)

                v1s = small.tile([64, 64], fp32, tag="v1s")
                nc.scalar.copy(v1s[:], v1[:])
                v2s = small.tile([64, 64], fp32, tag="v2s")
                nc.scalar.copy(v2s[:], v2[:])

            # ---- build W = [W_real | W_imag] (64,128) directly in fp16 ----
            # Mr[a,b] = S64[2a,2b]-S64[2a+1,2b+1], Mi = -(S64[2a,2b+1]+S64[2a+1,2b])
            # W_real = [[Mr, -Mi], [-Mi, -Mr]], W_imag = [[Mi, Mr], [Mr, -Mi]]
            # v1[0:32]=even rows, v1[32:64]=odd; v2 swapped. Scale is folded
            # into g on the host (GSCALE^2 == SCALE).
            wh = small.tile([64, 128], fp16, tag="wh")
            E, O = slice(0, 64, 2), slice(1, 64, 2)
            t, b = slice(0, 32), slice(32, 64)
            # top rows: Mr | -Mi(=mp) | Mi | Mr
            nc.vector.tensor_sub(wh[t, 0:32], v1s[t, E], v2s[t, O])
            nc.vector.tensor_add(wh[t, 32:64], v1s[t, O], v2s[t, E])
            nc.vector.scalar_tensor_tensor(wh[t, 64:96], v1s[t, O], -1.0,
                                           v2s[t, E], op0=mult, op1=sub_)
            nc.gpsimd.tensor_sub(wh[t, 96:128], v1s[t, E], v2s[t, O])
            # bottom rows: -Mi(=mp) | -Mr | Mr | -Mi(=mp)
            nc.vector.tensor_add(wh[b, 0:32], v2s[b, O], v1s[b, E])
            nc.vector.tensor_sub(wh[b, 32:64], v1s[b, O], v2s[b, E])
            nc.gpsimd.tensor_sub(wh[b, 64:96], v2s[b, E], v1s[b, O])
            nc.gpsimd.tensor_add(wh[b, 96:128], v2s[b, O], v1s[b, E])

            # PE warm-keepers bridge the epilogue gap
            with tc.tile_pool(name="wpsum", bufs=1, space="PSUM") as wpsum:
                warm_ps = wpsum.tile([64, 64], fp32, tag="warm")
                for w in range(4):
                    nc.tensor.matmul(warm_ps[:], lhsT=c_sb[:, 0:64],
                                     rhs=acc[:, 0:64], start=True, stop=True)

            # ---- phase 3: e = g .* (W^T g) chunks stream straight to DRAM;
            # the host does the final (tiny) column sums ----
            with tc.tile_pool(name="tpsum", bufs=6, space="PSUM") as tpsum:
                for ci in range(N_FCHUNK):
                    fs = slice(ci * FCHUNK, (ci + 1) * FCHUNK)
                    t_ps = tpsum.tile([128, FCHUNK], fp32, tag="t",
                                      name=f"t{ci}")
                    nc.tensor.matmul(t_ps[:], lhsT=wh[:], rhs=gth2[0:64, fs],
                                     start=True, stop=True)
                    e_sb = epool.tile([128, FCHUNK], fp16, tag="e",
                                      name=f"e{ci}")
                    if ci % 8 < 5:
                        nc.vector.tensor_mul(e_sb[:], gth2[:, fs], t_ps[:])
                    else:
                        # stage T to SBUF on Act so the Pool engine (no PSUM
                        # access) can handle half the elementwise work
                        t_sb = tsb_pool.tile([128, FCHUNK], fp16, tag="tsb",
                                             name=f"tsb{ci}")
                        nc.scalar.copy(t_sb[:], t_ps[:])
                        nc.gpsimd.tensor_mul(e_sb[:], gth2[:, fs], t_sb[:])
                    nc.sync.dma_start(out_d[:, fs], e_sb[:])

    nc.compile()
    return nc


def _prep_g(fbv):
    """gth (64, F) fp16: sqrt(SCALE) * [fbv_re.T; fbv_im.T]."""
    fbv32 = np.ascontiguousarray(fbv).view(np.float32).reshape(F, 2 * R)
    gbt = np.concatenate([fbv32[:, 0::2].T, fbv32[:, 1::2].T], axis=0)
    return (gbt * np.float32(GSCALE)).astype(np.float16)


def _shard_h(arr, core):
    """Core's complex64 shard -> fp16 (4, 128, 2*MACRO_COLS)."""
    sh = arr[core * DIR_PER_CORE:(core + 1) * DIR_PER_CORE]
    f32 = np.ascontiguousarray(sh).view(np.float32)
    return f32.astype(np.float16).reshape(4, 128, 2 * MACRO_COLS)


def _build_in_maps(attenuation_vectors, radiation_vectors,
                   frequency_basis_vectors):
    gth = _prep_g(frequency_basis_vectors)
    consts = _build_consts()
    in_maps = []
    for c in range(N_CORES):
        in_maps.append({
            "rad": _shard_h(radiation_vectors, c),
            "att": _shard_h(attenuation_vectors, c),
            "gth": gth,
            "consts": consts,
        })
    return in_maps


def kernel(attenuation_vectors, radiation_vectors, frequency_basis_vectors):
    from concourse.bass_utils import run_bass_kernel_spmd

    if "nc" not in _NC_CACHE:
        _NC_CACHE["nc"] = build_nc()
    nc = _NC_CACHE["nc"]

    in_maps = _build_in_maps(attenuation_vectors, radiation_vectors,
                             frequency_basis_vectors)
    res = run_bass_kernel_spmd(nc, in_maps, core_ids=list(range(N_CORES)))
    etot = np.zeros((128, F), np.float64)
    for r in res.results:
        etot += r["eout"].astype(np.float64)
    return (etot[0:64].sum(axis=0)
            + 1j * etot[64:128].sum(axis=0)).astype(np.complex64)
